# revision 36
# baseline (speedup 1.0000x reference)
"""Trainium2 Bass kernel for the masked note-accuracy loss.

Reference math (per sequence n):
    pred      = (sigmoid(x) > 0.5) = (x > 0)
    S_n       = sum_{t,d} pred * target                     (tru_pos)
    A[n,t]    = false_pos + false_neg = sum_d |pred - target|
    ratio     = S_n / (S_n + A[n,t]) = 2S_n / (2S_n + 2A[n,t])
    acc_n     = sum_{t<T_n} ratio / T_n,   T_n = sum_t mask[n,t]
    out       = sum_n acc_n
Sharding: data-parallel over N=128 sequences -> 16 per core on 8 cores;
the host sums the 8 per-core [128,16] partial tiles.

Per-core pipeline ([T,D] slabs loaded as [128,k,88] tiles, t = p*16+k;
x rides the SP HWDGE queue, y the ACT queue).  Per chunk:
  DVE p1: V = (x>0) - target (bf16), accum_out = per-partition (P-Q)
  GPSIMD: Copy(2*target), accum_out = per-partition 2Q  (keeps both
          DMA queues' sequencers free of compute waits)
  DVE p2: -A[t] cols = negated segmented abs-reduce of V over d
Per sequence (after its last chunk): PE colsums -> one psum row
[-A(16) | P-Q cols | 2Q cols]; its full reduce is directly 2S (sign
trick); den = -2*(-A) + 2S; rat = recip(den)*2S*mask' (1/T_n folded
into mask'); ratio colsums ACCUMULATE across sequences 0..14 in PSUM.

Tail shaping: sequences stream as k=8 halves; seq 14 as [8,4,4] and
seq 15 as a decreasing [6,4,4,2] cascade, so the DVE's processing lag
(~900ns DMA-sem + the chunk in flight) shrinks toward the stream's end
and only the final k=2 chunk's p1/p2 plus the epilogue chain is
exposed after the last input byte.  (A whole-tile tail exposes a full
3.4us p1/p2; chunking everything smaller starves the shared HWDGE
descriptor generator, which sustains ~1 DMA/628ns.)  Every load
allocates a full [P,KMAX,D] tile from ONE pool rotation and fills only
its sub-range -- the rotation's WAR chain paces all loads in exact
stream order (an unpaced tile's descriptor generation pre-runs and
steals earlier bus slots).  Seqs 13/14's epilogues run on the idle
ACT/Pool engines (emitted after all y-DMAs are issued so their data
waits cannot stall the ACT sequencer).  The mask is loaded once in raw
contiguous layout; T_i and the valid mask are rebuilt on-chip from an
iota-vs-T compare (== the reference's (t < T_i) semantics).  Final:
seq 15's rat tile ships as one [128,16] DMA the moment it is ready,
and the PSUM-accumulated 0..14 row ships separately on the idle ACT
queue; the host sums both.
"""

import numpy as np

import concourse.bacc as bacc
import concourse.tile as tile
from concourse import mybir
from concourse.alu_op_type import AluOpType
from concourse.bass_utils import run_bass_kernel_spmd

N, T, D = 128, 2048, 88
N_CORES = 8
NS = N // N_CORES
P = 128
K = T // P

_cached_nc = None

# chunk sizes per sequence (kc list, k0s derived); every chunk must fit
# the uniform [P, KMAX, D] tile rotation (kc <= KMAX).  Sequences 0..12
# stream whole ([16]); the tail cascade's decreasing sizes keep the
# DVE's lag behind the stream small when the last byte lands.
PLAN = {n: [8, 8] for n in range(14)}
PLAN[14] = [8, 4, 4]
PLAN[15] = [6, 4, 4, 2]
# sequences whose epilogue runs on ACT/Pool instead of DVE (they finish
# inside the tail cascade)
OFFLOAD_EPI = (13, 14)
LOOKAHEAD = 3
KMAX = 8
DATA_BUFS = 7
WORK_BUFS = 2
MINI_BUFS = 3


def _derive(plan):
    """Returns (CH, UNITS, COLB, NCOLS) for a chunk plan; COLB is the
    column base in stPQ/stQ2 for each sequence's chunks."""
    ch = {}
    for n in range(NS):
        k0 = 0
        ch[n] = []
        for kc in plan[n]:
            ch[n].append((k0, kc))
            k0 += kc
        assert k0 == K
    units = [(n, ci) for n in range(NS) for ci in range(len(ch[n]))]
    colb = {}
    c = 0
    for n in range(NS):
        colb[n] = c
        c += len(ch[n])
    return ch, units, colb, c


CH, UNITS, COLB, NCOLS = _derive(PLAN)


def _build():
    f32 = mybir.dt.float32
    vdt = mybir.dt.bfloat16
    nc = bacc.Bacc("TRN2", target_bir_lowering=False, debug=False,
                   num_devices=N_CORES)
    xd = nc.dram_tensor("output", [NS, T, D], f32, kind="ExternalInput")
    yd = nc.dram_tensor("target", [NS, T, D], f32, kind="ExternalInput")
    md = nc.dram_tensor("mask", [NS, T], mybir.dt.int32, kind="ExternalInput")
    od15 = nc.dram_tensor("partial15", [P, K], f32, kind="ExternalOutput")
    odacc = nc.dram_tensor("partial_acc", [1, K], f32, kind="ExternalOutput")

    AX = mybir.AxisListType.X

    with tile.TileContext(nc) as tc:
        with (
            tc.tile_pool(name="data", bufs=3) as data_pool,
            tc.tile_pool(name="work", bufs=WORK_BUFS) as work_pool,
            tc.tile_pool(name="mini", bufs=MINI_BUFS) as mini_pool,
            tc.tile_pool(name="singles", bufs=1) as singles,
            tc.tile_pool(name="psl", bufs=2, space="PSUM") as psum_loop,
            tc.tile_pool(name="psk", bufs=1, space="PSUM") as psum_keep,
        ):
            stA = singles.tile([P, NS, K], f32)
            stPQ = singles.tile([P, NCOLS], f32)
            stQ2 = singles.tile([P, NCOLS], f32)
            maskf = singles.tile([P, NS, K], f32)
            mraw = singles.tile([P, T * NS // P], mybir.dt.int32)
            mrawf = singles.tile([P, T * NS // P], f32)
            rowsum = singles.tile([P, 1], f32)
            blockind = singles.tile([P, NS], f32)
            id16 = singles.tile([NS, NS], f32)
            t16 = singles.tile([NS, 1], f32)
            iota_t = singles.tile([P, K], mybir.dt.int32)
            ones128 = singles.tile([P, P], f32)
            inv_ti = singles.tile([1, NS], f32)
            row_ti = singles.tile([1, NS], f32)
            iota_pn = singles.tile([P, NS], mybir.dt.int32)
            iota_mn = singles.tile([NS, NS], mybir.dt.int32)
            tmp_pn = singles.tile([P, NS], f32)
            nc.vector.memset(ones128[:], 1.0)
            # blockind[p, n] = 1 iff p//8 == n, i.e. 0 <= p-8n < 8
            nc.gpsimd.iota(iota_pn[:], pattern=[[-8, NS]], base=0,
                           channel_multiplier=1)
            nc.vector.tensor_scalar(
                out=tmp_pn[:], in0=iota_pn[:], scalar1=0.0, scalar2=None,
                op0=AluOpType.is_ge)
            tmp_pn2 = singles.tile([P, NS], f32)
            nc.vector.tensor_scalar(
                out=tmp_pn2[:], in0=iota_pn[:], scalar1=8.0, scalar2=None,
                op0=AluOpType.is_lt)
            nc.vector.tensor_mul(blockind[:], tmp_pn[:], tmp_pn2[:])
            # id16[m, n] = (m == n)
            nc.gpsimd.iota(iota_mn[:], pattern=[[-1, NS]], base=0,
                           channel_multiplier=1)
            nc.vector.tensor_scalar(
                out=id16[:], in0=iota_mn[:], scalar1=0.0, scalar2=None,
                op0=AluOpType.is_equal)
            nc.gpsimd.iota(iota_t[:], pattern=[[1, K]], base=0,
                           channel_multiplier=K)
            ps_t16 = psum_keep.tile([NS, 1], f32)
            ps_ti = psum_keep.tile([1, NS], f32)
            ps_tb = psum_keep.tile([P, NS], f32)
            ps_itb = psum_keep.tile([P, NS], f32)
            ps_acc = psum_keep.tile([P, K], f32)
            sb_tb = singles.tile([P, NS], f32)
            sb_itb = singles.tile([P, NS], f32)
            sb_acc = singles.tile([1, K], f32)

            def load_unit(u):
                # every unit allocates a full [P, KMAX, D] tile from ONE
                # rotation and fills only its [0:kc] sub-range: the pool's
                # WAR chain then paces ALL loads in exact stream order (a
                # fresh/unpaced tile would let its descriptor generation
                # pre-run and steal earlier bus slots)
                n, ci = u
                k0, kc = CH[n][ci]
                xt = data_pool.tile([P, KMAX, D], f32, tag="xu",
                                    bufs=DATA_BUFS, name="xt")
                yt = data_pool.tile([P, KMAX, D], f32, tag="yu",
                                    bufs=DATA_BUFS, name="yt")
                src = xd.ap()[n].rearrange("(p k) d -> p k d", p=P)
                nc.sync.dma_start(xt[:, 0:kc, :], src[:, k0 : k0 + kc, :])
                srcy = yd.ap()[n].rearrange("(p k) d -> p k d", p=P)
                nc.scalar.dma_start(yt[:, 0:kc, :], srcy[:, k0 : k0 + kc, :])
                return xt, yt

            def p1p2(u, xt, yt):
                n, ci = u
                k0, kc = CH[n][ci]
                col = COLB[n] + ci
                v = work_pool.tile([P, KMAX, D], vdt, tag="v", name="v")
                nc.vector.scalar_tensor_tensor(
                    out=v[:, 0:kc, :], in0=xt[:, 0:kc, :], scalar=0.0,
                    in1=yt[:, 0:kc, :],
                    op0=AluOpType.is_gt, op1=AluOpType.subtract,
                    accum_out=stPQ[:, col : col + 1],
                )
                nc.vector.tensor_reduce(
                    out=stA[:, n, k0 : k0 + kc], in_=v[:, 0:kc, :], axis=AX,
                    op=AluOpType.add, apply_absolute_value=True, negate=True,
                )
                scratch = work_pool.tile([P, KMAX, D], vdt, tag="s",
                                         name="scratch")
                nc.scalar.activation(
                    out=scratch[:, 0:kc, :], in_=yt[:, 0:kc, :],
                    func=mybir.ActivationFunctionType.Copy, scale=2.0,
                    accum_out=stQ2[:, col : col + 1],
                )

            def epilogue(n):
                ncn = len(CH[n])
                base = COLB[n]
                w = K + 2 * ncn
                maxc = max(len(CH[m]) for m in range(NS))
                ps_st = psum_loop.tile([P, K + 2 * maxc], f32, tag="ps_st",
                                       name="ps_st")
                # stPQ/stQ2 colsums first (ready before the last p2)
                nc.tensor.matmul(ps_st[:, K : K + ncn], ones128[:],
                                 stPQ[:, base : base + ncn])
                nc.tensor.matmul(ps_st[:, K + ncn : K + 2 * ncn], ones128[:],
                                 stQ2[:, base : base + ncn])
                nc.tensor.matmul(ps_st[:, 0:K], ones128[:], stA[:, n, :])
                s2p = mini_pool.tile([P, 1], f32, tag="s2p", name="s2p")
                den = mini_pool.tile([P, K], f32, tag="den", name="den")
                offload = n in OFFLOAD_EPI
                if offload:
                    # seqs 13/14 finish inside the tail cascade: run their
                    # 2S reduce, den and rat on the otherwise-idle ACT/Pool
                    # engines (emission is delayed until all y-DMAs are
                    # issued) so the DVE stays clear for seq 15's chain
                    junk = mini_pool.tile([P, K + 2 * maxc], f32, tag="junk",
                                          name="junk")
                    nc.scalar.activation(
                        out=junk[:, 0:w], in_=ps_st[:, 0:w],
                        func=mybir.ActivationFunctionType.Copy,
                        accum_out=s2p[:])
                    nc.scalar.activation(
                        out=den[:], in_=stA[:, n, :],
                        func=mybir.ActivationFunctionType.Identity,
                        scale=-2.0, bias=s2p[:])
                else:
                    nc.vector.tensor_reduce(
                        out=s2p[:], in_=ps_st[:, 0:w], axis=AX,
                        op=AluOpType.add)
                    nc.vector.tensor_scalar(
                        out=den[:], in0=stA[:, n, :],
                        scalar1=-2.0, scalar2=s2p[:], op0=AluOpType.mult,
                        op1=AluOpType.add)
                rec = mini_pool.tile([P, K], f32, tag="rec", name="rec")
                nc.vector.reciprocal(rec[:], den[:])
                rat = mini_pool.tile([P, K], f32, tag="rat", name="rat")
                nc.vector.scalar_tensor_tensor(
                    out=rat[:], in0=rec[:], scalar=s2p[:],
                    in1=maskf[:, n, :],
                    op0=AluOpType.mult, op1=AluOpType.mult)
                if n < NS - 1:
                    nc.tensor.matmul(ps_acc[:], ones128[:], rat[:],
                                     start=(n == 0), stop=(n == NS - 2))
                    if n == NS - 2:
                        # stage the accumulated 0..14 row into SBUF on the
                        # idle scalar engine and ship it early on the (now
                        # idle) ACT DMA queue; the host adds both outputs
                        nc.scalar.activation(
                            out=sb_acc[:], in_=ps_acc[0:1, :],
                            func=mybir.ActivationFunctionType.Copy)
                        nc.scalar.dma_start(odacc.ap(), sb_acc[:])
                else:
                    # seq 15's rat tile ships the moment it is ready
                    nc.sync.dma_start(od15.ap(), rat[:])

            # track how many chunks of each sequence have been processed so
            # the epilogue fires after the LAST stream unit of the sequence
            done = {n: 0 for n in range(NS)}

            # software pipeline: loads run LOOKAHEAD units ahead of compute
            tiles = {}
            for i in range(LOOKAHEAD):
                tiles[i] = load_unit(UNITS[i])
                if i == 0:
                    # mask chain rides the Pool SWDGE queue; T_i and the
                    # valid mask are rebuilt on-chip
                    nc.gpsimd.dma_start(
                        mraw[:], md.ap().rearrange("n (g j) -> (n g) j", g=8))
                    nc.vector.tensor_copy(mrawf[:], mraw[:])
                    nc.vector.tensor_reduce(out=rowsum[:], in_=mrawf[:],
                                            axis=AX, op=AluOpType.add)
                    nc.tensor.matmul(ps_t16[:], blockind[:], rowsum[:])
                    nc.vector.tensor_copy(t16[:], ps_t16[:])
                    nc.tensor.matmul(ps_ti[:], t16[:], id16[:])
                    nc.vector.tensor_copy(row_ti[:], ps_ti[:])
                    nc.vector.reciprocal(inv_ti[:], row_ti[:])
                    nc.tensor.matmul(ps_tb[:], ones128[0:1, :], row_ti[:])
                    nc.tensor.matmul(ps_itb[:], ones128[0:1, :], inv_ti[:])
                    nc.vector.tensor_copy(sb_tb[:], ps_tb[:])
                    nc.vector.tensor_copy(sb_itb[:], ps_itb[:])
                    # maskf[p,n,k] = (t < T_n) / T_n
                    for n in range(NS):
                        nc.vector.tensor_scalar(
                            out=maskf[:, n, :], in0=iota_t[:],
                            scalar1=sb_tb[:, n : n + 1],
                            scalar2=sb_itb[:, n : n + 1], op0=AluOpType.is_lt,
                            op1=AluOpType.mult)

            # seqs 13/14 use ACT-engine epilogues: delay their emission
            # until every y-DMA has been issued (an earlier ACT data-wait
            # would stall the ACT sequencer and the remaining y-DMA issue)
            release_at = len(UNITS) - 1 - LOOKAHEAD
            pending_epi = []
            for i, u in enumerate(UNITS):
                j = i + LOOKAHEAD
                if j < len(UNITS):
                    tiles[j] = load_unit(UNITS[j])
                xt, yt = tiles.pop(i)
                p1p2(u, xt, yt)
                if i >= release_at:
                    while pending_epi:
                        epilogue(pending_epi.pop(0))
                n, ci = u
                done[n] += 1
                if done[n] == len(CH[n]):
                    if n in OFFLOAD_EPI:
                        pending_epi.append(n)
                    else:
                        epilogue(n)

    nc.compile()
    return nc


def kernel(output, target, mask):
    global _cached_nc
    if _cached_nc is None:
        _cached_nc = _build()
    nc = _cached_nc
    output = np.asarray(output, dtype=np.float32)
    target = np.asarray(target, dtype=np.float32)
    mask = np.asarray(mask, dtype=np.int32)
    in_maps = []
    for c in range(N_CORES):
        sl = slice(c * NS, (c + 1) * NS)
        in_maps.append({
            "output": np.ascontiguousarray(output[sl]),
            "target": np.ascontiguousarray(target[sl]),
            "mask": np.ascontiguousarray(mask[sl]),
        })
    res = run_bass_kernel_spmd(nc, in_maps, list(range(N_CORES)))
    total = np.float32(0.0)
    for c in range(N_CORES):
        part = np.sum(res.results[c]["partial15"], dtype=np.float64)
        part += np.sum(res.results[c]["partial_acc"], dtype=np.float64)
        total = np.float32(total + np.float32(part))
    return np.float32(total)
